# revision 24
# baseline (speedup 1.0000x reference)
"""Trainium2 Bass kernel for nn_MultiHeadAttention_52261162058330.

Reference computes, per (batch, head):
    scores = X @ X.T          # [T, T]
    out    = scores @ X       # [T, D]
with X = x[b, h] of shape [T=2048, D=64], no softmax / no scaling.

Design:
 1. Associativity: out = (X X^T) X = X (X^T X) = X @ G with G = X^T X a
    [64, 64] Gram matrix -> ~32x fewer FLOPs.
 2. Pure bf16 (H = bf16(X)) with fp32 PSUM accumulation: end-to-end rel
    l2 error ~2.8e-3 (gate is 2e-2).  Dropping the old split-precision L
    path halves tensor/vector/scalar work.
 3. Pair-fused PE schedule, per pair q of row-tiles (u=2q, v=2q+1):
      pair = [H_u | H_v]                  [128(T), 128]   (stationary)
      MM_t: pst    = pair^T @ I128        -> [H_u^T; H_v^T]   (transpose)
      MM_g: psg   += pair^T @ pair        -> diag blocks accumulate
                                             G_even (p 0:64) / G_odd (p 64:128)
    Same stationary for both matmuls -> walrus --enable-ldw-opt=true drops
    the second LDWEIGHTS.
 4. Partition fold: G = G_even + G_odd duplicated onto both partition
    halves by ONE matmul with constant J2 = [[I,I],[I,I]]:
      psf[m, n] = sum_k J2[k, m] gsb[k, n] = gsb[m%64, n] + gsb[m%64+64, n]
 5. Out stage, one matmul per pair with block-diagonal rhs:
      pso = xt_q^T @ blockdiag(Gh, Gh)  -> [H_u Gh | H_v Gh]   [128, 128]
 6. Engine split: sync issues input DMAs (HWDGE ring 1), scalar issues
    output DMAs (HWDGE ring 2); the f32->bf16 input cast and the four
    PSUM evacuations per head are split across vector and scalar; gpsimd
    only builds the constant masks.  A post-compile pass drops duplicate
    PE Ldweights (walrus's --enable-ldw-opt crashes on this program).

Sharding: B*H = 32 (batch, head) pairs -> 4 heads per core on 8 cores,
fully independent (no collectives).

Layout per head (T split as (p u): partition p holds rows 16p..16p+15,
contiguous per partition for DMA).
"""

import numpy as np

N_CORES = 8
B, H, T, D = 2, 16, 2048, 64
HPC = (B * H) // N_CORES  # heads per core
U = T // 128              # 16 row-tiles per head
NP = U // 2               # 8 pairs per head

_NC = None


def _patch_walrus_flags():
    """Flip --enable-ldw-opt so walrus drops redundant LDWEIGHTS (the
    pair stage issues two matmuls per stationary)."""
    from concourse import bass_utils

    if getattr(bass_utils, "_ldw_patched", False):
        return
    orig = bass_utils.run_command

    def run_command(cmd, *a, **kw):
        if cmd and "walrus_driver" in str(cmd[0]):
            cmd = ["--enable-ldw-opt=true" if c == "--enable-ldw-opt=false" else c
                   for c in cmd]
        return orig(cmd, *a, **kw)

    bass_utils.run_command = run_command
    bass_utils._ldw_patched = True


def _patch_tile_tail():
    """Slim TileContext's exit sequence: drop the second all-engine barrier
    (only needed to fence re-entry, which a kernel tail doesn't have)."""
    from concourse import tile as tile_mod

    if getattr(tile_mod.TileContext, "_tail_patched", False):
        return
    from concourse.tile import ScopedClock

    def _drain_and_barrier(self, tick_clock, wait_clock):
        drain_inst = self.nc.sync.drain()
        wait_clock.add_sem_waits(
            drain_inst.ins, ScopedClock({None: tick_clock.global_clock})
        )
        self.nc.all_engine_barrier()
        popped = self.nc._tile_sem_poison_stack.pop()
        assert popped is self._sem_poison
        self.nc.clear_and_free_semaphores(list(self.sems.allocated().values()))

    tile_mod.TileContext._drain_and_barrier = _drain_and_barrier
    tile_mod.TileContext._tail_patched = True


def _dedup_ldweights(nc, mybir):
    """Drop PE Ldweights that reload the exact weights already resident
    (pair stage issues transpose+Gram matmuls off one stationary; walrus's
    own --enable-ldw-opt pass crashes on this program).  Only waitless,
    updateless loads are dropped, so semaphore bookkeeping is untouched."""
    import json

    n_dropped = 0
    for func in nc.m.functions:
        for blk in func.blocks:
            last_key = None
            keep = []
            for inst in blk.instructions:
                if getattr(inst, "engine", None) != mybir.EngineType.PE:
                    keep.append(inst)
                    continue
                if isinstance(inst, mybir.InstLdweights):
                    ij = json.loads(nc.instruction_to_json(inst))
                    key = json.dumps(
                        [ij.get("ins"), ij.get("is_transpose")], sort_keys=True,
                    )
                    if (key == last_key and not inst.has_wait()
                            and not inst.has_update()):
                        n_dropped += 1
                        continue
                    last_key = key
                elif isinstance(inst, mybir.InstMatmult):
                    pass  # uses resident weights, does not clobber them
                elif inst.is_sequencer_only():
                    pass  # sem ops / nops do not touch the PE array
                else:
                    last_key = None
                keep.append(inst)
            blk.instructions[:] = keep


def _build():
    import concourse.bacc as bacc
    import concourse.mybir as mybir
    from concourse import tile, masks

    _patch_tile_tail()

    nc = bacc.Bacc(
        trn_type="TRN2", target_bir_lowering=False, debug=False,
        num_devices=N_CORES,
    )
    f32 = mybir.dt.float32
    bf16 = mybir.dt.bfloat16
    x_in = nc.dram_tensor("x_shard", [HPC, T, D], f32, kind="ExternalInput").ap()
    y_out = nc.dram_tensor("out_shard", [HPC, T, D], f32, kind="ExternalOutput").ap()
    xv = x_in.rearrange("h (p u) d -> p h u d", p=128)
    yv = y_out.rearrange("h (p u) d -> p h u d", p=128)

    with tile.TileContext(nc) as tc:
        with (
            tc.tile_pool(name="const", bufs=1) as cpool,
            tc.tile_pool(name="xin", bufs=3) as xpool,
            tc.tile_pool(name="hbuf", bufs=3) as hpool,
            tc.tile_pool(name="xt", bufs=2) as tpool,
            tc.tile_pool(name="gsm", bufs=2) as gpool,
            tc.tile_pool(name="osb", bufs=3) as opool,
            tc.tile_pool(name="psT", bufs=2, space="PSUM") as psT,
            tc.tile_pool(name="psG", bufs=2, space="PSUM") as psG,
            tc.tile_pool(name="psF", bufs=1, space="PSUM") as psF,
            tc.tile_pool(name="psO", bufs=3, space="PSUM") as psO,
        ):
            identb = cpool.tile([128, 128], bf16)
            masks.make_identity(nc, identb[:])
            # blkmask[p, j, d] = 1 iff (p < 64) == (j == 0): selects the
            # block-diagonal slots of [128, 2, 64]
            blkmask = cpool.tile([128, 2, D], bf16)
            masks.make_block_diagonal(
                nc, blkmask.rearrange("p a b -> p (a b)"), D
            )
            # J2[k, m] = 1 iff k % 64 == m % 64  ([[I,I],[I,I]] stacked)
            j2 = cpool.tile([128, 128], bf16)
            nc.gpsimd.memset(j2[:], 0.0)
            for base in (0, -64, 64):
                nc.gpsimd.affine_select(
                    out=j2[:], in_=j2[:],
                    compare_op=mybir.AluOpType.not_equal,
                    fill=1.0, base=base,
                    pattern=[[-1, 128]], channel_multiplier=1,
                )

            wu = psF.tile([128, D], f32, tag="psf")
            for _ in range(48):
                nc.tensor.matmul(wu[0:64, :], identb[:, 0:64], identb[:, 0:64],
                                 start=True, stop=True)

            for h in range(HPC):
                xsb = xpool.tile([128, U, D], f32, tag="xsb")
                hb = hpool.tile([128, U, D], bf16, tag="hb")
                chunks = (0, 4, 8, U) if h == 0 else (0, 8, U)
                for c in range(len(chunks) - 1):
                    sl = slice(chunks[c], chunks[c + 1])
                    nc.sync.dma_start(out=xsb[:, sl], in_=xv[:, h, sl])
                    if c == 0:
                        nc.vector.tensor_copy(hb[:, sl], xsb[:, sl])
                    else:
                        nc.scalar.copy(hb[:, sl], xsb[:, sl])

                # pair stage: transpose + Gram, shared stationary per pair
                xt = tpool.tile([128, NP, 128], bf16, tag="xt")
                psg = psG.tile([128, 128], f32, tag="psg")
                for half in range(2):
                    pst = psT.tile([128, 4, 128], f32, tag="pst")
                    for i in range(4):
                        q = 4 * half + i
                        pair = hb[:, 2 * q:2 * q + 2].rearrange("p a b -> p (a b)")
                        nc.tensor.matmul(pst[:, i, :], pair, identb[:],
                                         start=True, stop=True)
                        nc.tensor.matmul(psg[:], pair, pair,
                                         start=(q == 0), stop=(q == NP - 1),
                                         skip_group_check=True)
                    if half == 0:
                        nc.scalar.copy(xt[:, 0:4, :], pst[:])
                    else:
                        nc.vector.tensor_copy(xt[:, 4:8, :], pst[:])

                # G = G_even + G_odd, duplicated to both partition halves
                gsb = gpool.tile([128, D], bf16, tag="gsb")
                nc.vector.tensor_copy(gsb[0:64, :], psg[0:64, 0:64])
                nc.vector.tensor_copy(gsb[64:128, :], psg[64:128, 64:128])
                psf = psF.tile([128, D], f32, tag="psf")
                nc.tensor.matmul(psf[:], j2[:], gsb[:], start=True, stop=True)
                g2blk = gpool.tile([128, 2, D], bf16, tag="g2blk")
                nc.vector.tensor_mul(
                    g2blk[:], psf[:][:, None, :].broadcast_to([128, 2, D]), blkmask[:]
                )
                g2m = g2blk.rearrange("p a b -> p (a b)")

                # out stage: one matmul per pair, rhs = blockdiag(Gh, Gh)
                osb = opool.tile([128, U, D], f32, tag="osb")
                for half in range(2):
                    pso = psO.tile([128, 4, 128], f32, tag="pso")
                    for i in range(4):
                        q = 4 * half + i
                        nc.tensor.matmul(pso[:, i, :], xt[:, q, :], g2m,
                                         start=True, stop=True)
                    osl = slice(8 * half, 8 * half + 8)
                    if half == 0:
                        nc.vector.tensor_copy(
                            osb[:, osl].rearrange("p a b -> p (a b)"),
                            pso[:].rearrange("p a b -> p (a b)"))
                    else:
                        nc.scalar.copy(
                            osb[:, osl].rearrange("p a b -> p (a b)"),
                            pso[:].rearrange("p a b -> p (a b)"))
                    nc.scalar.dma_start(out=yv[:, h, osl], in_=osb[:, osl])

    nc.compile()
    _dedup_ldweights(nc, mybir)
    return nc


def _get_nc():
    global _NC
    if _NC is None:
        _NC = _build()
    return _NC


def kernel(x: np.ndarray) -> np.ndarray:
    from concourse.bass_utils import run_bass_kernel_spmd

    assert x.shape == (B, H, T, D), x.shape
    x_flat = np.ascontiguousarray(x.reshape(B * H, T, D), dtype=np.float32)
    in_maps = [
        {"x_shard": np.ascontiguousarray(x_flat[c * HPC:(c + 1) * HPC])}
        for c in range(N_CORES)
    ]
    res = run_bass_kernel_spmd(_get_nc(), in_maps, list(range(N_CORES)))
    out = np.concatenate([res.results[c]["out_shard"] for c in range(N_CORES)], axis=0)
    return out.reshape(B, H, T, D)


# revision 26
# speedup vs baseline: 1.0279x; 1.0279x over previous
"""Trainium2 Bass kernel for nn_MultiHeadAttention_52261162058330.

Reference computes, per (batch, head):
    scores = X @ X.T          # [T, T]
    out    = scores @ X       # [T, D]
with X = x[b, h] of shape [T=2048, D=64], no softmax / no scaling.

Design:
 1. Associativity: out = (X X^T) X = X (X^T X) = X @ G with G = X^T X a
    [64, 64] Gram matrix -> ~32x fewer FLOPs.
 2. Pure bf16 (H = bf16(X)) with fp32 PSUM accumulation: end-to-end rel
    l2 error ~2.8e-3 (gate is 2e-2).  Dropping the old split-precision L
    path halves tensor/vector/scalar work.
 3. Pair-fused PE schedule, per pair q of row-tiles (u=2q, v=2q+1):
      pair = [H_u | H_v]                  [128(T), 128]   (stationary)
      MM_t: pst    = pair^T @ I128        -> [H_u^T; H_v^T]   (transpose)
      MM_g: psg   += pair^T @ pair        -> diag blocks accumulate
                                             G_even (p 0:64) / G_odd (p 64:128)
    Same stationary for both matmuls -> walrus --enable-ldw-opt=true drops
    the second LDWEIGHTS.
 4. Partition fold: G = G_even + G_odd duplicated onto both partition
    halves by ONE matmul with constant J2 = [[I,I],[I,I]]:
      psf[m, n] = sum_k J2[k, m] gsb[k, n] = gsb[m%64, n] + gsb[m%64+64, n]
 5. Out stage, one matmul per pair with block-diagonal rhs:
      pso = xt_q^T @ blockdiag(Gh, Gh)  -> [H_u Gh | H_v Gh]   [128, 128]
 6. Engine split: sync issues input DMAs (HWDGE ring 1), scalar issues
    output DMAs (HWDGE ring 2); the f32->bf16 input cast and the four
    PSUM evacuations per head are split across vector and scalar; gpsimd
    only builds the constant masks.  A post-compile pass drops duplicate
    PE Ldweights (walrus's --enable-ldw-opt crashes on this program).

Sharding: B*H = 32 (batch, head) pairs -> 4 heads per core on 8 cores,
fully independent (no collectives).

Layout per head (T split as (p u): partition p holds rows 16p..16p+15,
contiguous per partition for DMA).
"""

import numpy as np

N_CORES = 8
B, H, T, D = 2, 16, 2048, 64
HPC = (B * H) // N_CORES  # heads per core
U = T // 128              # 16 row-tiles per head
NP = U // 2               # 8 pairs per head

_NC = None


def _patch_walrus_flags():
    """Flip --enable-ldw-opt so walrus drops redundant LDWEIGHTS (the
    pair stage issues two matmuls per stationary)."""
    from concourse import bass_utils

    if getattr(bass_utils, "_ldw_patched", False):
        return
    orig = bass_utils.run_command

    def run_command(cmd, *a, **kw):
        if cmd and "walrus_driver" in str(cmd[0]):
            cmd = ["--enable-ldw-opt=true" if c == "--enable-ldw-opt=false" else c
                   for c in cmd]
        return orig(cmd, *a, **kw)

    bass_utils.run_command = run_command
    bass_utils._ldw_patched = True


def _patch_tile_tail():
    """Slim TileContext's exit sequence: drop the second all-engine barrier
    (only needed to fence re-entry, which a kernel tail doesn't have)."""
    from concourse import tile as tile_mod

    if getattr(tile_mod.TileContext, "_tail_patched", False):
        return
    from concourse.tile import ScopedClock

    def _drain_and_barrier(self, tick_clock, wait_clock):
        drain_inst = self.nc.sync.drain()
        wait_clock.add_sem_waits(
            drain_inst.ins, ScopedClock({None: tick_clock.global_clock})
        )
        self.nc.all_engine_barrier()
        popped = self.nc._tile_sem_poison_stack.pop()
        assert popped is self._sem_poison
        self.nc.clear_and_free_semaphores(list(self.sems.allocated().values()))

    tile_mod.TileContext._drain_and_barrier = _drain_and_barrier
    tile_mod.TileContext._tail_patched = True


def _dedup_ldweights(nc, mybir):
    """Drop PE Ldweights that reload the exact weights already resident
    (pair stage issues transpose+Gram matmuls off one stationary; walrus's
    own --enable-ldw-opt pass crashes on this program).  Only waitless,
    updateless loads are dropped, so semaphore bookkeeping is untouched."""
    import json

    n_dropped = 0
    for func in nc.m.functions:
        for blk in func.blocks:
            last_key = None
            keep = []
            for inst in blk.instructions:
                if getattr(inst, "engine", None) != mybir.EngineType.PE:
                    keep.append(inst)
                    continue
                if isinstance(inst, mybir.InstLdweights):
                    ij = json.loads(nc.instruction_to_json(inst))
                    key = json.dumps(
                        [ij.get("ins"), ij.get("is_transpose")], sort_keys=True,
                    )
                    if (key == last_key and not inst.has_wait()
                            and not inst.has_update()):
                        n_dropped += 1
                        continue
                    last_key = key
                elif isinstance(inst, mybir.InstMatmult):
                    pass  # uses resident weights, does not clobber them
                elif inst.is_sequencer_only():
                    pass  # sem ops / nops do not touch the PE array
                else:
                    last_key = None
                keep.append(inst)
            blk.instructions[:] = keep


def _build():
    import concourse.bacc as bacc
    import concourse.mybir as mybir
    from concourse import tile, masks

    _patch_tile_tail()

    nc = bacc.Bacc(
        trn_type="TRN2", target_bir_lowering=False, debug=False,
        num_devices=N_CORES,
    )
    f32 = mybir.dt.float32
    bf16 = mybir.dt.bfloat16
    x_in = nc.dram_tensor("x_shard", [HPC, T, D], f32, kind="ExternalInput").ap()
    y_out = nc.dram_tensor("out_shard", [HPC, T, D], f32, kind="ExternalOutput").ap()
    xv = x_in.rearrange("h (p u) d -> p h u d", p=128)
    yv = y_out.rearrange("h (p u) d -> p h u d", p=128)

    with tile.TileContext(nc) as tc:
        with (
            tc.tile_pool(name="const", bufs=1) as cpool,
            tc.tile_pool(name="xin", bufs=3) as xpool,
            tc.tile_pool(name="hbuf", bufs=3) as hpool,
            tc.tile_pool(name="xt", bufs=2) as tpool,
            tc.tile_pool(name="gsm", bufs=2) as gpool,
            tc.tile_pool(name="osb", bufs=3) as opool,
            tc.tile_pool(name="psT", bufs=2, space="PSUM") as psT,
            tc.tile_pool(name="psG", bufs=2, space="PSUM") as psG,
            tc.tile_pool(name="psF", bufs=1, space="PSUM") as psF,
            tc.tile_pool(name="psO", bufs=3, space="PSUM") as psO,
        ):
            identb = cpool.tile([128, 128], bf16)
            masks.make_identity(nc, identb[:])
            # blkmask[p, j, d] = 1 iff (p < 64) == (j == 0): selects the
            # block-diagonal slots of [128, 2, 64]
            blkmask = cpool.tile([128, 2, D], bf16)
            masks.make_block_diagonal(
                nc, blkmask.rearrange("p a b -> p (a b)"), D
            )
            # J2[k, m] = 1 iff k % 64 == m % 64  ([[I,I],[I,I]] stacked)
            j2 = cpool.tile([128, 128], bf16)
            nc.gpsimd.memset(j2[:], 0.0)
            for base in (0, -64, 64):
                nc.gpsimd.affine_select(
                    out=j2[:], in_=j2[:],
                    compare_op=mybir.AluOpType.not_equal,
                    fill=1.0, base=base,
                    pattern=[[-1, 128]], channel_multiplier=1,
                )

            wu = psF.tile([128, D], f32, tag="psf")
            for _ in range(48):
                nc.tensor.matmul(wu[0:64, :], identb[:, 0:64], identb[:, 0:64],
                                 start=True, stop=True)

            for h in range(HPC):
                xsb = xpool.tile([128, U, D], f32, tag="xsb")
                hb = hpool.tile([128, U, D], bf16, tag="hb")
                chunks = (0, 4, 8, U) if h == 0 else (0, 8, U)
                for c in range(len(chunks) - 1):
                    sl = slice(chunks[c], chunks[c + 1])
                    nc.sync.dma_start(out=xsb[:, sl], in_=xv[:, h, sl])
                    if c == 0:
                        nc.vector.tensor_copy(hb[:, sl], xsb[:, sl])
                    else:
                        nc.scalar.copy(hb[:, sl], xsb[:, sl])

                # pair stage: transpose + Gram, shared stationary per pair
                xt = tpool.tile([128, NP, 128], bf16, tag="xt")
                psg = psG.tile([128, 128], f32, tag="psg")
                for half in range(2):
                    pst = psT.tile([128, 4, 128], f32, tag="pst")
                    for i in range(4):
                        q = 4 * half + i
                        pair = hb[:, 2 * q:2 * q + 2].rearrange("p a b -> p (a b)")
                        nc.tensor.matmul(pst[:, i, :], pair, identb[:],
                                         start=True, stop=True)
                        nc.tensor.matmul(psg[:], pair, pair,
                                         start=(q == 0), stop=(q == NP - 1),
                                         skip_group_check=True)
                    if half == 0:
                        nc.scalar.copy(xt[:, 0:4, :], pst[:])
                    else:
                        nc.vector.tensor_copy(xt[:, 4:8, :], pst[:])

                # G = G_even + G_odd, duplicated to both partition halves
                gsb = gpool.tile([128, D], bf16, tag="gsb")
                nc.vector.tensor_copy(gsb[0:64, :], psg[0:64, 0:64])
                nc.vector.tensor_copy(gsb[64:128, :], psg[64:128, 64:128])
                psf = psF.tile([128, D], f32, tag="psf")
                nc.tensor.matmul(psf[:], j2[:], gsb[:], start=True, stop=True)
                g2blk = gpool.tile([128, 2, D], bf16, tag="g2blk")
                nc.vector.tensor_mul(
                    g2blk[:], psf[:][:, None, :].broadcast_to([128, 2, D]), blkmask[:]
                )
                g2m = g2blk.rearrange("p a b -> p (a b)")

                # out stage: one matmul per pair, rhs = blockdiag(Gh, Gh)
                osb = opool.tile([128, U, D], f32, tag="osb")
                for half in range(2):
                    pso = psO.tile([128, 4, 128], f32, tag="pso")
                    for i in range(4):
                        q = 4 * half + i
                        nc.tensor.matmul(pso[:, i, :], xt[:, q, :], g2m,
                                         start=True, stop=True)
                    osl = slice(8 * half, 8 * half + 8)
                    if half == 0:
                        nc.vector.tensor_copy(
                            osb[:, osl].rearrange("p a b -> p (a b)"),
                            pso[:].rearrange("p a b -> p (a b)"))
                    else:
                        nc.scalar.copy(
                            osb[:, osl].rearrange("p a b -> p (a b)"),
                            pso[:].rearrange("p a b -> p (a b)"))
                    nc.scalar.dma_start(out=yv[:, h, osl], in_=osb[:, osl])

    nc.compile()
    _dedup_ldweights(nc, mybir)
    return nc


def _get_nc():
    global _NC
    if _NC is None:
        _NC = _build()
    return _NC


def kernel(x: np.ndarray) -> np.ndarray:
    from concourse.bass_utils import run_bass_kernel_spmd

    assert x.shape == (B, H, T, D), x.shape
    x_flat = np.ascontiguousarray(x.reshape(B * H, T, D), dtype=np.float32)
    in_maps = [
        {"x_shard": np.ascontiguousarray(x_flat[c * HPC:(c + 1) * HPC])}
        for c in range(N_CORES)
    ]
    res = run_bass_kernel_spmd(_get_nc(), in_maps, list(range(N_CORES)))
    out = np.concatenate([res.results[c]["out_shard"] for c in range(N_CORES)], axis=0)
    return out.reshape(B, H, T, D)
